# revision 45
# baseline (speedup 1.0000x reference)
"""GQA kernel for Trainium2, 8 NeuronCores.

Problem: B=2, T=2048, HIDDEN=1024, 16 q-heads, 4 kv-heads, head_dim=64,
causal attention + output projection.

Sharding: core = (batch b = core//4, kv-group g = core%4). Each core handles
one batch element and the 4 query heads sharing kv-head g. o_proj is
ROW-parallel: each core contracts its own 4 heads (256 of 1024 attn dims)
against its 256 rows of Wo for ALL output columns, then a per-chunk bf16
ReduceScatter(add) across the batch group sums the partials and leaves each
core exactly its 256-column outT slice. No gather of attention outputs is
needed, so o_proj for chunk c runs immediately after chunk c is normalized
and overlaps the remaining attention chunks; only the last chunk's
ReduceScatter is tail-exposed.

Device dataflow (all matmuls bf16 with fp32 PSUM accumulation):
  - host supplies xT = x[b].T in bf16 ([1024, 2048]; hidden on partitions)
  - qT/kT via W-stationary matmuls (outputs transposed: head_dim on partitions)
  - V natural via PE transposes of vT tiles; ones column appended -> softmax
    denominators fall out of the PV matmul for free
  - S^T = kT.T @ qT directly (no transposes in the attention inner loop);
    2 heads packed per pass via PE row-tiling (K=64 each)
  - exp on ACT engine; diagonal tiles restrict matmul/exp to the valid causal
    q-range (single strided-AP exp across both packed heads) and mask only
    the 128-wide triangle slab (one bf16 0/1 multiply per head pair)
  - o^T_aug[65, Tq] accumulated per head in PSUM, evacuated to SBUF fast
    (frees PSUM for the next chunk); normalization trails off-path (approx
    reciprocal on DVE, broadcast on gpsimd)
  - per-chunk: partial o_proj (16 matmuls) -> bf16 staging -> DRAM ->
    ReduceScatter(add) into a Shared DRAM tile -> f32 upconvert to outT
  - output is outT [256, 2048] (rows 256g..256g+256 of out[b].T);
    host concatenates and transposes back.
"""

import sys

import numpy as np

try:
    import concourse.bass as bass
except ImportError:
    sys.path.insert(0, "/opt/trn_rl_repo")
    import concourse.bass as bass

import ml_dtypes
from contextlib import ExitStack

import concourse.tile as tile
from concourse import bacc, mybir
from concourse.bass import ds, ts
from concourse.bass_utils import run_bass_kernel_spmd
from concourse.masks import make_identity

BF16 = mybir.dt.bfloat16
F32 = mybir.dt.float32

P = 128
T = 2048
HID = 1024
KT = HID // P  # 8 k-tiles over hidden
CH = 512       # T_q chunk width
NCHUNK = T // CH
D = 64         # head dim
SCALE = D ** -0.5

_PROGRAM = None


def build_program():
    nc = bacc.Bacc(num_devices=8)

    # all inputs pre-shuffled on host so DMA reads are contiguous per
    # partition: xT[p, c, kt, t], wqkv[p, kt, n], wo[p, blk, n]
    xT_d = nc.declare_dram_parameter("xT", [P, NCHUNK * KT * CH], BF16, isOutput=False)
    wqkv_d = nc.declare_dram_parameter("wqkv", [P, KT * 384], BF16, isOutput=False)
    wo_d = nc.declare_dram_parameter("wo", [P, 2 * HID], BF16, isOutput=False)
    mask_d = nc.declare_dram_parameter("maskc", [P, 2 * P], BF16, isOutput=False)
    # chunk-major so each ReduceScatter writes one contiguous block; bf16 so
    # the rs_out->outT copy needs no cast (host upconverts for free)
    outT_d = nc.declare_dram_parameter("outT", [NCHUNK, 256, CH], BF16, isOutput=True)

    with tile.TileContext(nc) as tc, ExitStack() as ctx:
        sing = ctx.enter_context(tc.tile_pool(name="sing", bufs=1))
        work = ctx.enter_context(tc.tile_pool(name="work", bufs=2, space="PSUM"))
        accp = ctx.enter_context(tc.tile_pool(name="accp", bufs=4, space="PSUM"))
        ptp = ctx.enter_context(tc.tile_pool(name="ptp", bufs=4))
        outp = ctx.enter_context(tc.tile_pool(name="outp", bufs=3))
        nrmp = ctx.enter_context(tc.tile_pool(name="nrmp", bufs=8))
        oevp = ctx.enter_context(tc.tile_pool(name="oevp", bufs=10))
        agp = ctx.enter_context(tc.tile_pool(name="agp", bufs=2))
        psg = ctx.enter_context(tc.tile_pool(name="psg", bufs=2))
        dram = ctx.enter_context(tc.tile_pool(name="dram", bufs=1, space="DRAM"))

        partial_d = [
            dram.tile([HID, CH], BF16, name=f"partial{c}") for c in range(NCHUNK)
        ]
        rs_out = [
            dram.tile([256, CH], BF16, name=f"rs_out{c}") for c in range(NCHUNK)
        ]

        # --- loads needed before chunk-0 compute: wqkv, xT chunk 0, mask.
        # per-kt interleaved so the kt=0 matmul starts after two small DMAs
        # instead of waiting for both full transfers ---
        wqkv_sb = sing.tile([P, KT, 384], BF16)
        xT_sb = sing.tile([P, KT, T], BF16)
        for kt in range(KT):
            nc.sync.dma_start(
                wqkv_sb[:, kt, :], wqkv_d[:, ds(kt * 384, 384)]
            )
            nc.sync.dma_start(
                xT_sb[:, kt, ts(0, CH)], xT_d[:, ds(kt * CH, CH)]
            )
        maskc = sing.tile([P, 2, P], BF16)
        nc.sync.dma_start(maskc, mask_d[:, :].rearrange("p (b t) -> p b t", b=2))
        ident = sing.tile([P, P], BF16)
        make_identity(nc, ident)
        # --- deferred loads ---
        for c in range(1, NCHUNK):
            nc.sync.dma_start(
                xT_sb[:, :, ts(c, CH)],
                xT_d[:, ds(c * KT * CH, KT * CH)].rearrange(
                    "p (kt t) -> p kt t", kt=KT
                ),
            )
        wo_sb = sing.tile([P, 2, HID], BF16)
        nc.sync.dma_start(wo_sb, wo_d.rearrange("p (blk n) -> p blk n", blk=2))

        # blocks: 0 = qT heads (0,1); 1 = qT heads (2,3); 2 = [kT | vT]
        qkvT_sb = sing.tile([P, 3, T], BF16)
        kdup = sing.tile([P, T], BF16)        # kT duplicated on both partition halves
        vaug = sing.tile([P, 16, 66], BF16)   # V natural per T_k tile + ones col (64)
        nc.gpsimd.memset(vaug[:, :, 64:65], 1.0)

        # warmup collective: absorbs the comm-init barrier + first-collective
        # ring warmup (~12us) before any real ReduceScatter needs the links
        dummy_in = dram.tile([4, 64], BF16, name="dummy_in")
        dummy_out = dram.tile([1, 64], BF16, name="dummy_out")
        zz = sing.tile([4, 64], BF16)
        nc.gpsimd.memset(zz, 0.0)
        nc.sync.dma_start(dummy_in, zz)
        nc.gpsimd.collective_compute(
            "ReduceScatter",
            mybir.AluOpType.add,
            replica_groups=[[0, 1, 2, 3], [4, 5, 6, 7]],
            ins=[dummy_in.opt()],
            outs=[dummy_out.opt()],
        )

        def qkv_proj(c):
            cs = ts(c, CH)
            for blk in range(3):
                pj = work.tile([P, 1024], F32, tag="work", name=f"pj{c}_{blk}")
                for kt in range(KT):
                    nc.tensor.matmul(
                        pj[:, 0:CH],
                        wqkv_sb[:, kt, ts(blk, P)],
                        xT_sb[:, kt, cs],
                        start=(kt == 0),
                        stop=(kt == KT - 1),
                    )
                if blk < 2:
                    nc.vector.tensor_copy(qkvT_sb[:, blk, cs], pj[:, 0:CH])
                else:
                    nc.vector.tensor_copy(kdup[0:64, cs], pj[0:64, 0:CH])
                    nc.vector.tensor_copy(kdup[64:128, cs], pj[0:64, 0:CH])
                    nc.vector.tensor_copy(qkvT_sb[64:128, 2, cs], pj[64:128, 0:CH])

        def v_nat(c):
            for j in range(4 * c, 4 * c + 4):
                vps = work.tile([P, 64], BF16, tag="work", name=f"vps{j}")
                nc.tensor.transpose(
                    vps[:, 0:64], qkvT_sb[64:128, 2, ts(j, P)], ident[64:128, 64:128]
                )
                nc.vector.tensor_copy(vaug[:, j, 0:64], vps[:, 0:64])

        def attn(c, inserts=()):
            # inserts: [(j, callback)] — emit callback's instructions when
            # the loop reaches j (overlaps its deps with attention tail)
            ntk = 4 * (c + 1)
            oa = [
                accp.tile([P, CH], F32, tag="acc", name=f"oa{c}_{h}")
                for h in range(4)
            ]
            for j in range(ntk):
                for ij, cb in inserts:
                    if j == ij:
                        cb()
                r = j - 4 * c  # >= 0 on the block diagonal
                off = P * r if r >= 0 else 0
                w = CH - off
                for hp in range(2):
                    s2 = work.tile([P, 1024], F32, tag="work", name=f"s2_{c}_{j}_{hp}")
                    nc.tensor.matmul(
                        s2[:, ds(off, w)],
                        kdup[0:64, ts(j, P)],
                        qkvT_sb[0:64, hp, ds(CH * c + off, w)],
                        start=True,
                        stop=True,
                        tile_position=(0, 0),
                    )
                    nc.tensor.matmul(
                        s2[:, ds(CH + off, w)],
                        kdup[64:128, ts(j, P)],
                        qkvT_sb[64:128, hp, ds(CH * c + off, w)],
                        start=True,
                        stop=True,
                        tile_position=(64, 0),
                    )
                    pt = ptp.tile([P, 1024], BF16, tag="pt", name=f"pt{c}_{j}_{hp}")
                    if r >= 0:
                        s2v = s2.rearrange("p (b t) -> p b t", b=2)
                        ptv = pt.rearrange("p (b t) -> p b t", b=2)
                        nc.scalar.activation(
                            ptv[:, :, ds(off, w)],
                            s2v[:, :, ds(off, w)],
                            mybir.ActivationFunctionType.Exp,
                        )
                        nc.vector.tensor_mul(
                            ptv[:, :, ds(off, P)], ptv[:, :, ds(off, P)], maskc
                        )
                    else:
                        nc.scalar.activation(
                            pt, s2, mybir.ActivationFunctionType.Exp
                        )
                    for hh in range(2):
                        h = 2 * hp + hh
                        nc.tensor.matmul(
                            oa[h][0:65, ds(off, w)],
                            vaug[:, j, 0:65],
                            pt[:, ds(CH * hh + off, w)],
                            start=(j == 0),
                            stop=(j == ntk - 1),
                            skip_group_check=True,
                        )
            return oa

        def epilogue(c, oa):
            # normalize straight out of PSUM: denominator row -> approx
            # reciprocal -> broadcast -> one multiply per head reading the
            # accumulator directly (no evacuation pass); atst[:, blk, :]
            # holds this core's 4 heads' normalized attn^T (bf16)
            atst = agp.tile([P, 2, CH], BF16, tag="atst", name=f"atst{c}")
            for hp in (0, 1):
                # denominator copies on ACT (idle at chunk boundaries) keep
                # the vector queue free; two reciprocals land side by side
                # in one tile so a single broadcast serves the head pair
                rcp2 = nrmp.tile([1, 2 * CH], F32, tag="rcp", name=f"rcp{c}_{hp}")
                dnm2 = nrmp.tile([1, 2 * CH], F32, tag="dnm", name=f"dnm{c}_{hp}")
                for hh in (0, 1):
                    nc.scalar.activation(
                        dnm2[:, ds(CH * hh, CH)],
                        oa[2 * hp + hh][64:65, :],
                        mybir.ActivationFunctionType.Copy,
                    )
                nc.vector.reciprocal_approx_fast(rcp2, dnm2)
                rb2 = nrmp.tile([64, 2 * CH], F32, tag="rb", name=f"rb{c}_{hp}")
                nc.gpsimd.partition_broadcast(rb2, rcp2)
                for hh in (0, 1):
                    h = 2 * hp + hh
                    nc.vector.tensor_mul(
                        atst[ds(64 * (h % 2), 64), h // 2, :],
                        oa[h][0:64, :],
                        rb2[:, ds(CH * hh, CH)],
                    )
            return atst

        def o_proj(c, atst):
            # row-parallel partial o_proj: all 1024 output dims from this
            # core's 256 attn dims; f32 partials staged to DRAM, then
            # ReduceScatter(add) within the batch group writes straight
            # into this core's outT slice (no post-collective work)
            pstage = psg.tile([P, 4, 1024], BF16, tag="pstage", name=f"pstage{c}")
            for mbp in range(4):
                ps = work.tile([P, 1024], F32, tag="work", name=f"ps{c}_{mbp}")
                for sub in range(2):
                    for blk in range(2):
                        nc.tensor.matmul(
                            ps[:, ts(sub, CH)],
                            wo_sb[:, blk, ts(2 * mbp + sub, P)],
                            atst[:, blk, :],
                            start=(blk == 0),
                            stop=(blk == 1),
                        )
                if mbp % 2 == 0:
                    nc.vector.tensor_copy(pstage[:, mbp, :], ps)
                else:
                    # alternate staging casts onto ACT so the work-pool
                    # rotation isn't gated by one serialized vector queue
                    nc.scalar.activation(
                        pstage[:, mbp, :], ps, mybir.ActivationFunctionType.Copy
                    )
            # stage in halves: the first DMA overlaps the second half's casts
            pd = partial_d[c].rearrange(
                "(mbp sub p) t -> p mbp sub t", p=P, sub=2
            )
            pstg = pstage.rearrange("p mbp (sub t) -> p mbp sub t", sub=2)
            nc.sync.dma_start(pd[:, 0:2], pstg[:, 0:2])
            nc.sync.dma_start(pd[:, 2:4], pstg[:, 2:4])
            nc.gpsimd.collective_compute(
                "ReduceScatter",
                mybir.AluOpType.add,
                replica_groups=[[0, 1, 2, 3], [4, 5, 6, 7]],
                ins=[partial_d[c].opt()],
                outs=[rs_out[c].opt()],
            )

        qkv_proj(0)
        carry = None  # o_proj of the previous chunk, inserted at j=1 so the
        # PE covers the normalization chain with early attention work
        for c in range(NCHUNK):
            ntk = 4 * (c + 1)
            inserts = []
            if carry is not None:
                inserts.append((3, carry))
            if c < NCHUNK - 1:
                # next chunk's qkv projection inside this chunk's tail so
                # its PSUM evacuations overlap attention
                inserts.append(
                    (max(3, ntk - 4), lambda cc=c + 1: qkv_proj(cc))
                )
            if c == 0:
                v_nat(0)
            oa = attn(c, inserts)
            if c < NCHUNK - 1:
                # emit next chunk's V transposes before the epilogue so
                # their vaug copies aren't queued behind the mul chain
                v_nat(c + 1)
            atst = epilogue(c, oa)
            carry = (lambda cc=c, aa=atst: o_proj(cc, aa))
        carry()

        # ---- copy ReduceScatter results to outT (plain sync DMAs; keeps
        # collective-gated work off the gpsimd broadcast queue) ----
        for c in range(NCHUNK):
            nc.sync.dma_start(outT_d[c], rs_out[c])

    nc.finalize()
    return nc


def _prep_inputs(x, Wq, Wkv, Wo):
    bf = ml_dtypes.bfloat16
    x = np.asarray(x, dtype=np.float32)
    Wq = np.asarray(Wq, dtype=np.float32)
    Wkv = np.asarray(Wkv, dtype=np.float32)
    Wo = np.asarray(Wo, dtype=np.float32)

    # lower-triangle mask, duplicated for the two packed heads:
    # M[r, b, qi] = 1.0 iff r <= qi
    tri = (np.arange(P)[:, None] <= np.arange(P)[None, :])
    mask = np.concatenate([tri, tri], axis=1).astype(bf)

    # pre-shuffle for contiguous per-partition DMA:
    # xT[p, c*KT*CH + kt*CH + t] = x[b].T[kt*P + p, c*CH + t]
    xT = []
    for b in range(2):
        xb = x[b].T.reshape(KT, P, NCHUNK, CH)          # [kt, p, c, t]
        xb = np.ascontiguousarray(xb.transpose(1, 2, 0, 3)).reshape(P, -1)
        xT.append(xb.astype(bf))

    in_maps = []
    for core in range(8):
        b, g = core // 4, core % 4
        wq_g = Wq[:, 256 * g : 256 * (g + 1)] * SCALE
        wk_g = Wkv[:, 64 * g : 64 * (g + 1)]
        wv_g = Wkv[:, 256 + 64 * g : 256 + 64 * (g + 1)]
        wqkv = np.concatenate([wq_g, wk_g, wv_g], axis=1)  # [1024, 384]
        # wqkv[p, kt*384 + n] = wqkv[kt*P + p, n]
        wqkv = np.ascontiguousarray(
            wqkv.reshape(KT, P, 384).transpose(1, 0, 2)
        ).reshape(P, -1).astype(bf)
        # wo[p, blk*HID + n] = Wo[256g + blk*P + p, n]
        wo_g = Wo[256 * g : 256 * (g + 1), :]
        wo_g = np.ascontiguousarray(
            wo_g.reshape(2, P, HID).transpose(1, 0, 2)
        ).reshape(P, -1).astype(bf)
        in_maps.append(
            {"xT": xT[b], "wqkv": wqkv, "wo": wo_g, "maskc": mask}
        )
    return in_maps


def run(x, Wq, Wkv, Wo, trace=False, **trace_kwargs):
    global _PROGRAM
    if _PROGRAM is None:
        _PROGRAM = build_program()
    nc = _PROGRAM
    in_maps = _prep_inputs(x, Wq, Wkv, Wo)
    res = run_bass_kernel_spmd(
        nc, in_maps, core_ids=list(range(8)), trace=trace, **trace_kwargs
    )
    outs = res.results
    full = np.empty((2, T, HID), dtype=np.float32)
    for b in range(2):
        outT_b = np.concatenate(
            [
                np.transpose(
                    np.asarray(outs[4 * b + g]["outT"]).astype(np.float32),
                    (1, 0, 2),
                ).reshape(256, T)
                for g in range(4)
            ],
            axis=0,
        )  # [1024, 2048]
        full[b] = outT_b.T
    return full, res


def kernel(x, Wq, Wkv, Wo):
    out, _ = run(x, Wq, Wkv, Wo, trace=False)
    return out


# revision 48
# speedup vs baseline: 1.0734x; 1.0734x over previous
"""GQA kernel for Trainium2, 8 NeuronCores.

Problem: B=2, T=2048, HIDDEN=1024, 16 q-heads, 4 kv-heads, head_dim=64,
causal attention + output projection.

Sharding: core = (batch b = core//4, kv-group g = core%4). Each core handles
one batch element and the 4 query heads sharing kv-head g. o_proj is
ROW-parallel: each core contracts its own 4 heads (256 of 1024 attn dims)
against its 256 rows of Wo for ALL output columns, then a per-chunk bf16
ReduceScatter(add) across the batch group sums the partials and leaves each
core exactly its 256-column outT slice. No gather of attention outputs is
needed, so o_proj for chunk c runs immediately after chunk c is normalized
and overlaps the remaining attention chunks; only the last chunk's
ReduceScatter is tail-exposed.

Device dataflow (all matmuls bf16 with fp32 PSUM accumulation):
  - host supplies xT = x[b].T in bf16 ([1024, 2048]; hidden on partitions)
  - qT/kT via W-stationary matmuls (outputs transposed: head_dim on partitions)
  - V natural via PE transposes of vT tiles; ones column appended -> softmax
    denominators fall out of the PV matmul for free
  - S^T = kT.T @ qT directly (no transposes in the attention inner loop);
    2 heads packed per pass via PE row-tiling (K=64 each)
  - exp on ACT engine; diagonal tiles restrict matmul/exp to the valid causal
    q-range (single strided-AP exp across both packed heads) and mask only
    the 128-wide triangle slab (one bf16 0/1 multiply per head pair)
  - o^T_aug[65, Tq] accumulated per head in PSUM, evacuated to SBUF fast
    (frees PSUM for the next chunk); normalization trails off-path (approx
    reciprocal on DVE, broadcast on gpsimd)
  - per-chunk: partial o_proj (16 matmuls) -> bf16 staging -> DRAM ->
    ReduceScatter(add) into a Shared DRAM tile -> f32 upconvert to outT
  - output is outT [256, 2048] (rows 256g..256g+256 of out[b].T);
    host concatenates and transposes back.
"""

import sys

import numpy as np

try:
    import concourse.bass as bass
except ImportError:
    sys.path.insert(0, "/opt/trn_rl_repo")
    import concourse.bass as bass

import ml_dtypes
from contextlib import ExitStack

import concourse.tile as tile
from concourse import bacc, mybir
from concourse.bass import ds, ts
from concourse.bass_utils import run_bass_kernel_spmd
from concourse.masks import make_identity

BF16 = mybir.dt.bfloat16
F32 = mybir.dt.float32

P = 128
T = 2048
HID = 1024
KT = HID // P  # 8 k-tiles over hidden
CH = 512       # T_q chunk width
NCHUNK = T // CH
D = 64         # head dim
SCALE = D ** -0.5

_PROGRAM = None


def build_program():
    nc = bacc.Bacc(num_devices=8)

    # all inputs pre-shuffled on host so DMA reads are contiguous per
    # partition: xT[p, c, kt, t], wqkv[p, kt, n], wo[p, blk, n]
    xT_d = nc.declare_dram_parameter("xT", [P, NCHUNK * KT * CH], BF16, isOutput=False)
    wqkv_d = nc.declare_dram_parameter("wqkv", [P, KT * 384], BF16, isOutput=False)
    wo_d = nc.declare_dram_parameter("wo", [P, 2 * HID], BF16, isOutput=False)
    mask_d = nc.declare_dram_parameter("maskc", [P, 2 * P], BF16, isOutput=False)
    # chunk-major so each ReduceScatter writes one contiguous block; bf16 so
    # the rs_out->outT copy needs no cast (host upconverts for free)
    outT_d = nc.declare_dram_parameter("outT", [NCHUNK, 256, CH], BF16, isOutput=True)

    with tile.TileContext(nc) as tc, ExitStack() as ctx:
        sing = ctx.enter_context(tc.tile_pool(name="sing", bufs=1))
        work = ctx.enter_context(tc.tile_pool(name="work", bufs=2, space="PSUM"))
        accp = ctx.enter_context(tc.tile_pool(name="accp", bufs=4, space="PSUM"))
        ptp = ctx.enter_context(tc.tile_pool(name="ptp", bufs=6))
        outp = ctx.enter_context(tc.tile_pool(name="outp", bufs=3))
        nrmp = ctx.enter_context(tc.tile_pool(name="nrmp", bufs=8))
        oevp = ctx.enter_context(tc.tile_pool(name="oevp", bufs=10))
        agp = ctx.enter_context(tc.tile_pool(name="agp", bufs=2))
        psg = ctx.enter_context(tc.tile_pool(name="psg", bufs=2))
        dram = ctx.enter_context(tc.tile_pool(name="dram", bufs=1, space="DRAM"))

        partial_d = [
            dram.tile([HID, CH], BF16, name=f"partial{c}") for c in range(NCHUNK)
        ]
        rs_out = [
            dram.tile([256, CH], BF16, name=f"rs_out{c}") for c in range(NCHUNK)
        ]

        # --- loads needed before chunk-0 compute: wqkv, xT chunk 0, mask.
        # per-kt interleaved so the kt=0 matmul starts after two small DMAs
        # instead of waiting for both full transfers ---
        wqkv_sb = sing.tile([P, KT, 384], BF16)
        xT_sb = sing.tile([P, KT, T], BF16)
        for kt in range(KT):
            nc.sync.dma_start(
                wqkv_sb[:, kt, :], wqkv_d[:, ds(kt * 384, 384)]
            )
            nc.sync.dma_start(
                xT_sb[:, kt, ts(0, CH)], xT_d[:, ds(kt * CH, CH)]
            )
        maskc = sing.tile([P, 2, P], BF16)
        nc.sync.dma_start(maskc, mask_d[:, :].rearrange("p (b t) -> p b t", b=2))
        ident = sing.tile([P, P], BF16)
        make_identity(nc, ident)
        # --- deferred loads ---
        for c in range(1, NCHUNK):
            nc.sync.dma_start(
                xT_sb[:, :, ts(c, CH)],
                xT_d[:, ds(c * KT * CH, KT * CH)].rearrange(
                    "p (kt t) -> p kt t", kt=KT
                ),
            )
        wo_sb = sing.tile([P, 2, HID], BF16)
        nc.sync.dma_start(wo_sb, wo_d.rearrange("p (blk n) -> p blk n", blk=2))

        # blocks: 0 = qT heads (0,1); 1 = qT heads (2,3); 2 = [kT | vT]
        qkvT_sb = sing.tile([P, 3, T], BF16)
        kdup = sing.tile([P, T], BF16)        # kT duplicated on both partition halves
        vaug = sing.tile([P, 16, 66], BF16)   # V natural per T_k tile + ones col (64)
        nc.gpsimd.memset(vaug[:, :, 64:65], 1.0)

        # warmup collective: absorbs the comm-init barrier + first-collective
        # ring warmup (~12us) before any real ReduceScatter needs the links
        dummy_in = dram.tile([4, 64], BF16, name="dummy_in")
        dummy_out = dram.tile([1, 64], BF16, name="dummy_out")
        zz = sing.tile([4, 64], BF16)
        nc.gpsimd.memset(zz, 0.0)
        nc.sync.dma_start(dummy_in, zz)
        nc.gpsimd.collective_compute(
            "ReduceScatter",
            mybir.AluOpType.add,
            replica_groups=[[0, 1, 2, 3], [4, 5, 6, 7]],
            ins=[dummy_in.opt()],
            outs=[dummy_out.opt()],
        )

        def qkv_proj(c):
            cs = ts(c, CH)
            for blk in range(3):
                pj = work.tile([P, 1024], F32, tag="work", name=f"pj{c}_{blk}")
                for kt in range(KT):
                    nc.tensor.matmul(
                        pj[:, 0:CH],
                        wqkv_sb[:, kt, ts(blk, P)],
                        xT_sb[:, kt, cs],
                        start=(kt == 0),
                        stop=(kt == KT - 1),
                    )
                if blk < 2:
                    nc.vector.tensor_copy(qkvT_sb[:, blk, cs], pj[:, 0:CH])
                else:
                    nc.vector.tensor_copy(kdup[0:64, cs], pj[0:64, 0:CH])
                    nc.vector.tensor_copy(kdup[64:128, cs], pj[0:64, 0:CH])
                    nc.vector.tensor_copy(qkvT_sb[64:128, 2, cs], pj[64:128, 0:CH])

        def v_nat(c):
            for j in range(4 * c, 4 * c + 4):
                vps = work.tile([P, 64], BF16, tag="work", name=f"vps{j}")
                nc.tensor.transpose(
                    vps[:, 0:64], qkvT_sb[64:128, 2, ts(j, P)], ident[64:128, 64:128]
                )
                nc.vector.tensor_copy(vaug[:, j, 0:64], vps[:, 0:64])

        def attn(c, inserts=()):
            # inserts: [(j, callback)] — emit callback's instructions when
            # the loop reaches j (overlaps its deps with attention tail)
            ntk = 4 * (c + 1)
            oa = [
                accp.tile([P, CH], F32, tag="acc", name=f"oa{c}_{h}")
                for h in range(4)
            ]
            deferred_pv = []
            for j in range(ntk):
                for ij, cb in inserts:
                    if j == ij:
                        cb()
                r = j - 4 * c  # >= 0 on the block diagonal
                off = P * r if r >= 0 else 0
                w = CH - off
                for hp in range(2):
                    s2 = work.tile([P, 1024], F32, tag="work", name=f"s2_{c}_{j}_{hp}")
                    nc.tensor.matmul(
                        s2[:, ds(off, w)],
                        kdup[0:64, ts(j, P)],
                        qkvT_sb[0:64, hp, ds(CH * c + off, w)],
                        start=True,
                        stop=True,
                        tile_position=(0, 0),
                    )
                    nc.tensor.matmul(
                        s2[:, ds(CH + off, w)],
                        kdup[64:128, ts(j, P)],
                        qkvT_sb[64:128, hp, ds(CH * c + off, w)],
                        start=True,
                        stop=True,
                        tile_position=(64, 0),
                    )
                    pt = ptp.tile([P, 1024], BF16, tag="pt", name=f"pt{c}_{j}_{hp}")
                    if r >= 0:
                        s2v = s2.rearrange("p (b t) -> p b t", b=2)
                        ptv = pt.rearrange("p (b t) -> p b t", b=2)
                        nc.scalar.activation(
                            ptv[:, :, ds(off, w)],
                            s2v[:, :, ds(off, w)],
                            mybir.ActivationFunctionType.Exp,
                        )
                        nc.vector.tensor_mul(
                            ptv[:, :, ds(off, P)], ptv[:, :, ds(off, P)], maskc
                        )
                    else:
                        nc.scalar.activation(
                            pt, s2, mybir.ActivationFunctionType.Exp
                        )

                    def emit_pv(jj, hhp, ppt, ooff, ww):
                        for hh in range(2):
                            h = 2 * hhp + hh
                            nc.tensor.matmul(
                                oa[h][0:65, ds(ooff, ww)],
                                vaug[:, jj, 0:65],
                                ppt[:, ds(CH * hh + ooff, ww)],
                                start=(jj == 0),
                                stop=(jj == ntk - 1),
                                skip_group_check=True,
                            )

                    if c > 0 and j < 2:
                        # software-pipelined prologue: defer j0/j1 PVs so the
                        # in-order PE queue issues independent S work while
                        # the previous chunk's normalize-muls release the oa
                        # accumulator banks (avoids head-of-line blocking)
                        deferred_pv.append((j, hp, pt, off, w))
                    else:
                        emit_pv(j, hp, pt, off, w)
                if c > 0 and j == 1:
                    for args in deferred_pv:
                        emit_pv(*args)
                    deferred_pv = []
            return oa

        def epilogue(c, oa):
            # normalize straight out of PSUM: denominator row -> approx
            # reciprocal -> broadcast -> one multiply per head reading the
            # accumulator directly (no evacuation pass); atst[:, blk, :]
            # holds this core's 4 heads' normalized attn^T (bf16)
            atst = agp.tile([P, 2, CH], BF16, tag="atst", name=f"atst{c}")
            for hp in (0, 1):
                # denominator copies on ACT (idle at chunk boundaries) keep
                # the vector queue free; two reciprocals land side by side
                # in one tile so a single broadcast serves the head pair
                rcp2 = nrmp.tile([1, 2 * CH], F32, tag="rcp", name=f"rcp{c}_{hp}")
                dnm2 = nrmp.tile([1, 2 * CH], F32, tag="dnm", name=f"dnm{c}_{hp}")
                for hh in (0, 1):
                    nc.scalar.activation(
                        dnm2[:, ds(CH * hh, CH)],
                        oa[2 * hp + hh][64:65, :],
                        mybir.ActivationFunctionType.Copy,
                    )
                nc.vector.reciprocal_approx_fast(rcp2, dnm2)
                rb2 = nrmp.tile([64, 2 * CH], F32, tag="rb", name=f"rb{c}_{hp}")
                nc.gpsimd.partition_broadcast(rb2, rcp2)
                for hh in (0, 1):
                    h = 2 * hp + hh
                    nc.vector.tensor_mul(
                        atst[ds(64 * (h % 2), 64), h // 2, :],
                        oa[h][0:64, :],
                        rb2[:, ds(CH * hh, CH)],
                    )
            return atst

        def o_proj(c, atst):
            # row-parallel partial o_proj: all 1024 output dims from this
            # core's 256 attn dims; f32 partials staged to DRAM, then
            # ReduceScatter(add) within the batch group writes straight
            # into this core's outT slice (no post-collective work)
            pstage = psg.tile([P, 4, 1024], BF16, tag="pstage", name=f"pstage{c}")
            for mbp in range(4):
                ps = work.tile([P, 1024], F32, tag="work", name=f"ps{c}_{mbp}")
                for sub in range(2):
                    for blk in range(2):
                        nc.tensor.matmul(
                            ps[:, ts(sub, CH)],
                            wo_sb[:, blk, ts(2 * mbp + sub, P)],
                            atst[:, blk, :],
                            start=(blk == 0),
                            stop=(blk == 1),
                        )
                if mbp % 2 == 0:
                    nc.vector.tensor_copy(pstage[:, mbp, :], ps)
                else:
                    # alternate staging casts onto ACT so the work-pool
                    # rotation isn't gated by one serialized vector queue
                    nc.scalar.activation(
                        pstage[:, mbp, :], ps, mybir.ActivationFunctionType.Copy
                    )
            # stage in halves: the first DMA overlaps the second half's casts
            pd = partial_d[c].rearrange(
                "(mbp sub p) t -> p mbp sub t", p=P, sub=2
            )
            pstg = pstage.rearrange("p mbp (sub t) -> p mbp sub t", sub=2)
            nc.sync.dma_start(pd[:, 0:2], pstg[:, 0:2])
            nc.sync.dma_start(pd[:, 2:4], pstg[:, 2:4])
            nc.gpsimd.collective_compute(
                "ReduceScatter",
                mybir.AluOpType.add,
                replica_groups=[[0, 1, 2, 3], [4, 5, 6, 7]],
                ins=[partial_d[c].opt()],
                outs=[rs_out[c].opt()],
            )

        qkv_proj(0)
        carry = None  # o_proj of the previous chunk, inserted at j=1 so the
        # PE covers the normalization chain with early attention work
        for c in range(NCHUNK):
            ntk = 4 * (c + 1)
            inserts = []
            if carry is not None:
                inserts.append((3, carry))
            if c < NCHUNK - 1:
                # next chunk's qkv projection inside this chunk's tail so
                # its PSUM evacuations overlap attention
                inserts.append(
                    (max(3, ntk - 4), lambda cc=c + 1: qkv_proj(cc))
                )
            if c == 0:
                v_nat(0)
            oa = attn(c, inserts)
            if c < NCHUNK - 1:
                # emit next chunk's V transposes before the epilogue so
                # their vaug copies aren't queued behind the mul chain
                v_nat(c + 1)
            atst = epilogue(c, oa)
            carry = (lambda cc=c, aa=atst: o_proj(cc, aa))
        carry()

        # ---- copy ReduceScatter results to outT (plain sync DMAs; keeps
        # collective-gated work off the gpsimd broadcast queue) ----
        for c in range(NCHUNK):
            nc.sync.dma_start(outT_d[c], rs_out[c])

    nc.finalize()
    return nc


def _prep_inputs(x, Wq, Wkv, Wo):
    bf = ml_dtypes.bfloat16
    x = np.asarray(x, dtype=np.float32)
    Wq = np.asarray(Wq, dtype=np.float32)
    Wkv = np.asarray(Wkv, dtype=np.float32)
    Wo = np.asarray(Wo, dtype=np.float32)

    # lower-triangle mask, duplicated for the two packed heads:
    # M[r, b, qi] = 1.0 iff r <= qi
    tri = (np.arange(P)[:, None] <= np.arange(P)[None, :])
    mask = np.concatenate([tri, tri], axis=1).astype(bf)

    # pre-shuffle for contiguous per-partition DMA:
    # xT[p, c*KT*CH + kt*CH + t] = x[b].T[kt*P + p, c*CH + t]
    xT = []
    for b in range(2):
        xb = x[b].T.reshape(KT, P, NCHUNK, CH)          # [kt, p, c, t]
        xb = np.ascontiguousarray(xb.transpose(1, 2, 0, 3)).reshape(P, -1)
        xT.append(xb.astype(bf))

    in_maps = []
    for core in range(8):
        b, g = core // 4, core % 4
        wq_g = Wq[:, 256 * g : 256 * (g + 1)] * SCALE
        wk_g = Wkv[:, 64 * g : 64 * (g + 1)]
        wv_g = Wkv[:, 256 + 64 * g : 256 + 64 * (g + 1)]
        wqkv = np.concatenate([wq_g, wk_g, wv_g], axis=1)  # [1024, 384]
        # wqkv[p, kt*384 + n] = wqkv[kt*P + p, n]
        wqkv = np.ascontiguousarray(
            wqkv.reshape(KT, P, 384).transpose(1, 0, 2)
        ).reshape(P, -1).astype(bf)
        # wo[p, blk*HID + n] = Wo[256g + blk*P + p, n]
        wo_g = Wo[256 * g : 256 * (g + 1), :]
        wo_g = np.ascontiguousarray(
            wo_g.reshape(2, P, HID).transpose(1, 0, 2)
        ).reshape(P, -1).astype(bf)
        in_maps.append(
            {"xT": xT[b], "wqkv": wqkv, "wo": wo_g, "maskc": mask}
        )
    return in_maps


def run(x, Wq, Wkv, Wo, trace=False, **trace_kwargs):
    global _PROGRAM
    if _PROGRAM is None:
        _PROGRAM = build_program()
    nc = _PROGRAM
    in_maps = _prep_inputs(x, Wq, Wkv, Wo)
    res = run_bass_kernel_spmd(
        nc, in_maps, core_ids=list(range(8)), trace=trace, **trace_kwargs
    )
    outs = res.results
    full = np.empty((2, T, HID), dtype=np.float32)
    for b in range(2):
        outT_b = np.concatenate(
            [
                np.transpose(
                    np.asarray(outs[4 * b + g]["outT"]).astype(np.float32),
                    (1, 0, 2),
                ).reshape(256, T)
                for g in range(4)
            ],
            axis=0,
        )  # [1024, 2048]
        full[b] = outT_b.T
    return full, res


def kernel(x, Wq, Wkv, Wo):
    out, _ = run(x, Wq, Wkv, Wo, trace=False)
    return out


# revision 49
# speedup vs baseline: 1.0833x; 1.0092x over previous
"""GQA kernel for Trainium2, 8 NeuronCores.

Problem: B=2, T=2048, HIDDEN=1024, 16 q-heads, 4 kv-heads, head_dim=64,
causal attention + output projection.

Sharding: core = (batch b = core//4, kv-group g = core%4). Each core handles
one batch element and the 4 query heads sharing kv-head g. o_proj is
ROW-parallel: each core contracts its own 4 heads (256 of 1024 attn dims)
against its 256 rows of Wo for ALL output columns, then a per-chunk bf16
ReduceScatter(add) across the batch group sums the partials and leaves each
core exactly its 256-column outT slice. No gather of attention outputs is
needed, so o_proj for chunk c runs immediately after chunk c is normalized
and overlaps the remaining attention chunks; only the last chunk's
ReduceScatter is tail-exposed.

Device dataflow (all matmuls bf16 with fp32 PSUM accumulation):
  - host supplies xT = x[b].T in bf16 ([1024, 2048]; hidden on partitions)
  - qT/kT via W-stationary matmuls (outputs transposed: head_dim on partitions)
  - V natural via PE transposes of vT tiles; ones column appended -> softmax
    denominators fall out of the PV matmul for free
  - S^T = kT.T @ qT directly (no transposes in the attention inner loop);
    2 heads packed per pass via PE row-tiling (K=64 each)
  - exp on ACT engine; diagonal tiles restrict matmul/exp to the valid causal
    q-range (single strided-AP exp across both packed heads) and mask only
    the 128-wide triangle slab (one bf16 0/1 multiply per head pair)
  - o^T_aug[65, Tq] accumulated per head in PSUM, evacuated to SBUF fast
    (frees PSUM for the next chunk); normalization trails off-path (approx
    reciprocal on DVE, broadcast on gpsimd)
  - per-chunk: partial o_proj (16 matmuls) -> bf16 staging -> DRAM ->
    ReduceScatter(add) into a Shared DRAM tile -> f32 upconvert to outT
  - output is outT [256, 2048] (rows 256g..256g+256 of out[b].T);
    host concatenates and transposes back.
"""

import sys

import numpy as np

try:
    import concourse.bass as bass
except ImportError:
    sys.path.insert(0, "/opt/trn_rl_repo")
    import concourse.bass as bass

import ml_dtypes
from contextlib import ExitStack

import concourse.tile as tile
from concourse import bacc, mybir
from concourse.bass import ds, ts
from concourse.bass_utils import run_bass_kernel_spmd
from concourse.masks import make_identity

BF16 = mybir.dt.bfloat16
F32 = mybir.dt.float32

P = 128
T = 2048
HID = 1024
KT = HID // P  # 8 k-tiles over hidden
CH = 512       # T_q chunk width
NCHUNK = T // CH
D = 64         # head dim
SCALE = D ** -0.5

_PROGRAM = None


def build_program():
    nc = bacc.Bacc(num_devices=8)

    # all inputs pre-shuffled on host so DMA reads are contiguous per
    # partition: xT[p, c, kt, t], wqkv[p, kt, n], wo[p, blk, n]
    xT_d = nc.declare_dram_parameter("xT", [P, NCHUNK * KT * CH], BF16, isOutput=False)
    wqkv_d = nc.declare_dram_parameter("wqkv", [P, KT * 384], BF16, isOutput=False)
    wo_d = nc.declare_dram_parameter("wo", [P, 2 * HID], BF16, isOutput=False)
    mask_d = nc.declare_dram_parameter("maskc", [P, 2 * P], BF16, isOutput=False)
    # chunk-major so each ReduceScatter writes one contiguous block; bf16 so
    # the rs_out->outT copy needs no cast (host upconverts for free)
    outT_d = nc.declare_dram_parameter("outT", [NCHUNK, 256, CH], BF16, isOutput=True)

    with tile.TileContext(nc) as tc, ExitStack() as ctx:
        sing = ctx.enter_context(tc.tile_pool(name="sing", bufs=1))
        work = ctx.enter_context(tc.tile_pool(name="work", bufs=2, space="PSUM"))
        accp = ctx.enter_context(tc.tile_pool(name="accp", bufs=4, space="PSUM"))
        ptp = ctx.enter_context(tc.tile_pool(name="ptp", bufs=4))
        outp = ctx.enter_context(tc.tile_pool(name="outp", bufs=3))
        nrmp = ctx.enter_context(tc.tile_pool(name="nrmp", bufs=8))
        oevp = ctx.enter_context(tc.tile_pool(name="oevp", bufs=10))
        agp = ctx.enter_context(tc.tile_pool(name="agp", bufs=2))
        psg = ctx.enter_context(tc.tile_pool(name="psg", bufs=2))
        dram = ctx.enter_context(tc.tile_pool(name="dram", bufs=1, space="DRAM"))

        partial_d = [
            dram.tile([HID, CH], BF16, name=f"partial{c}") for c in range(NCHUNK)
        ]
        rs_out = [
            dram.tile([256, CH], BF16, name=f"rs_out{c}") for c in range(NCHUNK)
        ]

        # --- loads needed before chunk-0 compute: wqkv, xT chunk 0, mask.
        # per-kt interleaved so the kt=0 matmul starts after two small DMAs
        # instead of waiting for both full transfers ---
        wqkv_sb = sing.tile([P, KT, 384], BF16)
        xT_sb = sing.tile([P, KT, T], BF16)
        for kt in range(KT):
            nc.sync.dma_start(
                wqkv_sb[:, kt, :], wqkv_d[:, ds(kt * 384, 384)]
            )
            nc.sync.dma_start(
                xT_sb[:, kt, ts(0, CH)], xT_d[:, ds(kt * CH, CH)]
            )
        maskc = sing.tile([P, 2, P], BF16)
        nc.sync.dma_start(maskc, mask_d[:, :].rearrange("p (b t) -> p b t", b=2))
        ident = sing.tile([P, P], BF16)
        make_identity(nc, ident)
        # --- deferred loads ---
        for c in range(1, NCHUNK):
            nc.sync.dma_start(
                xT_sb[:, :, ts(c, CH)],
                xT_d[:, ds(c * KT * CH, KT * CH)].rearrange(
                    "p (kt t) -> p kt t", kt=KT
                ),
            )
        wo_sb = sing.tile([P, 2, HID], BF16)
        nc.sync.dma_start(wo_sb, wo_d.rearrange("p (blk n) -> p blk n", blk=2))

        # blocks: 0 = qT heads (0,1); 1 = qT heads (2,3); 2 = [kT | vT]
        qkvT_sb = sing.tile([P, 3, T], BF16)
        kdup = sing.tile([P, T], BF16)        # kT duplicated on both partition halves
        vaug = sing.tile([P, 16, 66], BF16)   # V natural per T_k tile + ones col (64)
        nc.gpsimd.memset(vaug[:, :, 64:65], 1.0)

        # warmup collective: absorbs the comm-init barrier + first-collective
        # ring warmup (~12us) before any real ReduceScatter needs the links
        dummy_in = dram.tile([4, 64], BF16, name="dummy_in")
        dummy_out = dram.tile([1, 64], BF16, name="dummy_out")
        zz = sing.tile([4, 64], BF16)
        nc.gpsimd.memset(zz, 0.0)
        nc.sync.dma_start(dummy_in, zz)
        nc.gpsimd.collective_compute(
            "ReduceScatter",
            mybir.AluOpType.add,
            replica_groups=[[0, 1, 2, 3], [4, 5, 6, 7]],
            ins=[dummy_in.opt()],
            outs=[dummy_out.opt()],
        )

        def qkv_proj(c):
            cs = ts(c, CH)
            for blk in range(3):
                pj = work.tile([P, 1024], F32, tag="work", name=f"pj{c}_{blk}")
                for kt in range(KT):
                    nc.tensor.matmul(
                        pj[:, 0:CH],
                        wqkv_sb[:, kt, ts(blk, P)],
                        xT_sb[:, kt, cs],
                        start=(kt == 0),
                        stop=(kt == KT - 1),
                    )
                if blk < 2:
                    nc.vector.tensor_copy(qkvT_sb[:, blk, cs], pj[:, 0:CH])
                else:
                    nc.vector.tensor_copy(kdup[0:64, cs], pj[0:64, 0:CH])
                    nc.vector.tensor_copy(kdup[64:128, cs], pj[0:64, 0:CH])
                    nc.vector.tensor_copy(qkvT_sb[64:128, 2, cs], pj[64:128, 0:CH])

        def v_nat(c):
            for j in range(4 * c, 4 * c + 4):
                vps = work.tile([P, 64], BF16, tag="work", name=f"vps{j}")
                nc.tensor.transpose(
                    vps[:, 0:64], qkvT_sb[64:128, 2, ts(j, P)], ident[64:128, 64:128]
                )
                nc.vector.tensor_copy(vaug[:, j, 0:64], vps[:, 0:64])

        def attn(c, inserts=()):
            # inserts: [(j, callback)] — emit callback's instructions when
            # the loop reaches j (overlaps its deps with attention tail)
            ntk = 4 * (c + 1)
            oa = [
                accp.tile([P, CH], F32, tag="acc", name=f"oa{c}_{h}")
                for h in range(4)
            ]
            for j in range(ntk):
                for ij, cb in inserts:
                    if j == ij:
                        cb()
                r = j - 4 * c  # >= 0 on the block diagonal
                off = P * r if r >= 0 else 0
                w = CH - off
                for hp in range(2):
                    s2 = work.tile([P, 1024], F32, tag="work", name=f"s2_{c}_{j}_{hp}")
                    nc.tensor.matmul(
                        s2[:, ds(off, w)],
                        kdup[0:64, ts(j, P)],
                        qkvT_sb[0:64, hp, ds(CH * c + off, w)],
                        start=True,
                        stop=True,
                        tile_position=(0, 0),
                    )
                    nc.tensor.matmul(
                        s2[:, ds(CH + off, w)],
                        kdup[64:128, ts(j, P)],
                        qkvT_sb[64:128, hp, ds(CH * c + off, w)],
                        start=True,
                        stop=True,
                        tile_position=(64, 0),
                    )
                    pt = ptp.tile([P, 1024], BF16, tag="pt", name=f"pt{c}_{j}_{hp}")
                    if r >= 0:
                        s2v = s2.rearrange("p (b t) -> p b t", b=2)
                        ptv = pt.rearrange("p (b t) -> p b t", b=2)
                        nc.scalar.activation(
                            ptv[:, :, ds(off, w)],
                            s2v[:, :, ds(off, w)],
                            mybir.ActivationFunctionType.Exp,
                        )
                        nc.vector.tensor_mul(
                            ptv[:, :, ds(off, P)], ptv[:, :, ds(off, P)], maskc
                        )
                    else:
                        nc.scalar.activation(
                            pt, s2, mybir.ActivationFunctionType.Exp
                        )
                    for hh in range(2):
                        h = 2 * hp + hh
                        nc.tensor.matmul(
                            oa[h][0:65, ds(off, w)],
                            vaug[:, j, 0:65],
                            pt[:, ds(CH * hh + off, w)],
                            start=(j == 0),
                            stop=(j == ntk - 1),
                            skip_group_check=True,
                        )
            return oa

        def epilogue(c, oa):
            # normalize straight out of PSUM: denominator row -> approx
            # reciprocal -> broadcast -> one multiply per head reading the
            # accumulator directly (no evacuation pass); atst[:, blk, :]
            # holds this core's 4 heads' normalized attn^T (bf16)
            atst = agp.tile([P, 2, CH], BF16, tag="atst", name=f"atst{c}")
            for hp in (0, 1):
                # denominator copies on ACT (idle at chunk boundaries) keep
                # the vector queue free; two reciprocals land side by side
                # in one tile so a single broadcast serves the head pair
                rcp2 = nrmp.tile([1, 2 * CH], F32, tag="rcp", name=f"rcp{c}_{hp}")
                dnm2 = nrmp.tile([1, 2 * CH], F32, tag="dnm", name=f"dnm{c}_{hp}")
                for hh in (0, 1):
                    nc.scalar.activation(
                        dnm2[:, ds(CH * hh, CH)],
                        oa[2 * hp + hh][64:65, :],
                        mybir.ActivationFunctionType.Copy,
                    )
                nc.vector.reciprocal_approx_fast(rcp2, dnm2)
                rb2 = nrmp.tile([64, 2 * CH], F32, tag="rb", name=f"rb{c}_{hp}")
                nc.gpsimd.partition_broadcast(rb2, rcp2)
                for hh in (0, 1):
                    h = 2 * hp + hh
                    nc.vector.tensor_mul(
                        atst[ds(64 * (h % 2), 64), h // 2, :],
                        oa[h][0:64, :],
                        rb2[:, ds(CH * hh, CH)],
                    )
            return atst

        def o_proj(c, atst):
            # row-parallel partial o_proj: all 1024 output dims from this
            # core's 256 attn dims; f32 partials staged to DRAM, then
            # ReduceScatter(add) within the batch group writes straight
            # into this core's outT slice (no post-collective work)
            pstage = psg.tile([P, 4, 1024], BF16, tag="pstage", name=f"pstage{c}")
            for mbp in range(4):
                ps = work.tile([P, 1024], F32, tag="work", name=f"ps{c}_{mbp}")
                for sub in range(2):
                    for blk in range(2):
                        nc.tensor.matmul(
                            ps[:, ts(sub, CH)],
                            wo_sb[:, blk, ts(2 * mbp + sub, P)],
                            atst[:, blk, :],
                            start=(blk == 0),
                            stop=(blk == 1),
                        )
                if mbp % 2 == 0:
                    nc.vector.tensor_copy(pstage[:, mbp, :], ps)
                else:
                    # alternate staging casts onto ACT so the work-pool
                    # rotation isn't gated by one serialized vector queue
                    nc.scalar.activation(
                        pstage[:, mbp, :], ps, mybir.ActivationFunctionType.Copy
                    )
            # stage in halves: the first DMA overlaps the second half's casts
            pd = partial_d[c].rearrange(
                "(mbp sub p) t -> p mbp sub t", p=P, sub=2
            )
            pstg = pstage.rearrange("p mbp (sub t) -> p mbp sub t", sub=2)
            nc.sync.dma_start(pd[:, 0:2], pstg[:, 0:2])
            nc.sync.dma_start(pd[:, 2:4], pstg[:, 2:4])
            nc.gpsimd.collective_compute(
                "ReduceScatter",
                mybir.AluOpType.add,
                replica_groups=[[0, 1, 2, 3], [4, 5, 6, 7]],
                ins=[partial_d[c].opt()],
                outs=[rs_out[c].opt()],
            )

        qkv_proj(0)
        carry = None  # o_proj of the previous chunk, inserted at j=1 so the
        # PE covers the normalization chain with early attention work
        for c in range(NCHUNK):
            ntk = 4 * (c + 1)
            inserts = []
            if carry is not None:
                inserts.append((3, carry))
            if c < NCHUNK - 1:
                # next chunk's qkv projection inside this chunk's tail so
                # its PSUM evacuations overlap attention
                inserts.append(
                    (max(3, ntk - 4), lambda cc=c + 1: qkv_proj(cc))
                )
            if c == 0:
                v_nat(0)
            oa = attn(c, inserts)
            if c < NCHUNK - 1:
                # emit next chunk's V transposes before the epilogue so
                # their vaug copies aren't queued behind the mul chain
                v_nat(c + 1)
            atst = epilogue(c, oa)
            carry = (lambda cc=c, aa=atst: o_proj(cc, aa))
        carry()

        # ---- copy ReduceScatter results to outT (plain sync DMAs; keeps
        # collective-gated work off the gpsimd broadcast queue) ----
        for c in range(NCHUNK):
            nc.sync.dma_start(outT_d[c], rs_out[c])

    nc.finalize()
    return nc


def _prep_inputs(x, Wq, Wkv, Wo):
    bf = ml_dtypes.bfloat16
    x = np.asarray(x, dtype=np.float32)
    Wq = np.asarray(Wq, dtype=np.float32)
    Wkv = np.asarray(Wkv, dtype=np.float32)
    Wo = np.asarray(Wo, dtype=np.float32)

    # lower-triangle mask, duplicated for the two packed heads:
    # M[r, b, qi] = 1.0 iff r <= qi
    tri = (np.arange(P)[:, None] <= np.arange(P)[None, :])
    mask = np.concatenate([tri, tri], axis=1).astype(bf)

    # pre-shuffle for contiguous per-partition DMA:
    # xT[p, c*KT*CH + kt*CH + t] = x[b].T[kt*P + p, c*CH + t]
    xT = []
    for b in range(2):
        xb = x[b].T.reshape(KT, P, NCHUNK, CH)          # [kt, p, c, t]
        xb = np.ascontiguousarray(xb.transpose(1, 2, 0, 3)).reshape(P, -1)
        xT.append(xb.astype(bf))

    in_maps = []
    for core in range(8):
        b, g = core // 4, core % 4
        wq_g = Wq[:, 256 * g : 256 * (g + 1)] * SCALE
        wk_g = Wkv[:, 64 * g : 64 * (g + 1)]
        wv_g = Wkv[:, 256 + 64 * g : 256 + 64 * (g + 1)]
        wqkv = np.concatenate([wq_g, wk_g, wv_g], axis=1)  # [1024, 384]
        # wqkv[p, kt*384 + n] = wqkv[kt*P + p, n]
        wqkv = np.ascontiguousarray(
            wqkv.reshape(KT, P, 384).transpose(1, 0, 2)
        ).reshape(P, -1).astype(bf)
        # wo[p, blk*HID + n] = Wo[256g + blk*P + p, n]
        wo_g = Wo[256 * g : 256 * (g + 1), :]
        wo_g = np.ascontiguousarray(
            wo_g.reshape(2, P, HID).transpose(1, 0, 2)
        ).reshape(P, -1).astype(bf)
        in_maps.append(
            {"xT": xT[b], "wqkv": wqkv, "wo": wo_g, "maskc": mask}
        )
    return in_maps


def run(x, Wq, Wkv, Wo, trace=False, **trace_kwargs):
    global _PROGRAM
    if _PROGRAM is None:
        _PROGRAM = build_program()
    nc = _PROGRAM
    in_maps = _prep_inputs(x, Wq, Wkv, Wo)
    res = run_bass_kernel_spmd(
        nc, in_maps, core_ids=list(range(8)), trace=trace, **trace_kwargs
    )
    outs = res.results
    full = np.empty((2, T, HID), dtype=np.float32)
    for b in range(2):
        outT_b = np.concatenate(
            [
                np.transpose(
                    np.asarray(outs[4 * b + g]["outT"]).astype(np.float32),
                    (1, 0, 2),
                ).reshape(256, T)
                for g in range(4)
            ],
            axis=0,
        )  # [1024, 2048]
        full[b] = outT_b.T
    return full, res


def kernel(x, Wq, Wkv, Wo):
    out, _ = run(x, Wq, Wkv, Wo, trace=False)
    return out
